# revision 7
# baseline (speedup 1.0000x reference)
"""MoE positionwise FFN (top-2 of 8 experts) on 8 TRN2 NeuronCores.

Strategy: expert-parallel, host-routed. The router (logits -> top-2 ->
softmax gates) is exact fp32 on host (as is the final scatter-add
combine, matching the reference semantics). Each core owns one expert:
the host gathers that expert's routed tokens into a compact bf16
input, and the device kernel is a pure dense FFN:

    h = relu(w1 @ x + b1)   (D -> F)
    y = w2 @ h + b2         (F -> D)

over C tokens in blocks (first block largest so block-0 compute
consumes the weight stream slower than DMA supplies it), weight-
stationary bf16 matmuls, fp32 PSUM accumulation.

All device inputs are pre-permuted on host into the exact SBUF layout
so every DMA is a contiguous slice (>=8KB per-partition runs, full HBM
bandwidth): w1 as [128, F, NTD] (f-sliced chunks stream while block-0
computes), w2 as [128, D, NTF] (d-sliced), x per block as [128, NTD,
TB]. Weight chunks alternate across both HWDGE rings (sync + scalar).
The matmul lhsT reads these with a small free-dim stride, which
LDWEIGHTS tolerates (stays hidden behind the matmuls). The PE is
pre-warmed so the HAM clock gate is at 8/8 when real matmuls start.

Self-contained: hardcodes shapes for B=2,T=2048,D=1024,F=4096,E=8,K=2.
"""
import math

import numpy as np
import ml_dtypes

S = 4096
D = 1024
F = 4096
E = 8
NTD = D // 128   # 8 d-tiles
NTF = F // 128   # 32 f-tiles

_cache: dict = {}
LAST_RES = None


def _plan_blocks(maxcnt: int):
    C = max(192, int(math.ceil(maxcnt / 16)) * 16)
    s0 = min(448, C)
    rem = C - s0
    if rem == 0:
        return (s0,)
    s1 = int(math.ceil(rem / 2 / 16)) * 16
    s2 = rem - s1
    if s2 == 0:
        return (s0, s1)
    return (s0, s1, s2)


def _build(blocks: tuple):
    import concourse.bacc as bacc
    import concourse.tile as tile
    import concourse.mybir as mybir
    from concourse.tile import add_dep_helper

    f32 = mybir.dt.float32
    bf16 = mybir.dt.bfloat16
    ACT = mybir.ActivationFunctionType

    NB = len(blocks)

    nc = bacc.Bacc("TRN2", target_bir_lowering=False, debug=False, num_devices=8)

    xg_d = [nc.dram_tensor(f"xg{b}", [128, NTD, blocks[b]], bf16,
                           kind="ExternalInput") for b in range(NB)]
    w1_d = nc.dram_tensor("w1f", [128, F, NTD], bf16, kind="ExternalInput")
    w2_d = nc.dram_tensor("w2f", [128, D, NTF], bf16, kind="ExternalInput")
    b1_d = nc.dram_tensor("b1r", [128, NTF], f32, kind="ExternalInput")
    b2_d = nc.dram_tensor("b2r", [128, NTD], f32, kind="ExternalInput")
    yg_d = [nc.dram_tensor(f"yg{b}", [128, NTD, blocks[b]], bf16,
                           kind="ExternalOutput") for b in range(NB)]

    with tile.TileContext(nc) as tc:
        with (
            tc.tile_pool(name="wpool", bufs=1) as wpool,
            tc.tile_pool(name="xr", bufs=1) as xr,
            tc.tile_pool(name="small", bufs=1) as small,
            tc.tile_pool(name="hpool", bufs=1) as hpool,
            tc.tile_pool(name="ypool", bufs=2) as ypool,
            tc.tile_pool(name="psH", bufs=3, space="PSUM") as psH,
            tc.tile_pool(name="psY", bufs=2, space="PSUM") as psY,
            tc.tile_pool(name="psW", bufs=1, space="PSUM") as psW,
        ):
            # ---- PE pre-warm: trip the HAM activity window during the
            # initial DMA so real matmuls start at 2.4 GHz.
            junk = small.tile([128, 256], bf16)
            nc.vector.memset(junk[:], 0.0)
            wps = psW.tile([128, 256], f32)
            for _ in range(12):
                nc.tensor.matmul(wps[:], lhsT=junk[:, 0:128], rhs=junk[:],
                                 start=True, stop=True)

            # ---- DMA streams on the two HWDGE rings.
            # ring B (scalar): xg block0 + biases, then odd weight chunks
            #                  (+ xg block2).
            # ring A (sync):   even weight chunks (+ xg block1); y-out
            #                  rides sync after everything.
            ringA = []  # sync
            ringB = []  # scalar

            def put(ring, fn):
                d = fn()
                if ring:
                    add_dep_helper(d.ins, ring[-1].ins, sync=False,
                                   reason="ring order")
                ring.append(d)
                return d

            xg = [xr.tile([128, NTD, blocks[b]], bf16, tag=f"xg{b}",
                          name=f"xg{b}")
                  for b in range(NB)]
            put(ringB, lambda: nc.scalar.dma_start(out=xg[0][:], in_=xg_d[0][:, :, :]))
            b1_sb = small.tile([128, NTF], f32)
            put(ringB, lambda: nc.scalar.dma_start(out=b1_sb[:], in_=b1_d[:, :]))
            b2_sb = small.tile([128, NTD], f32)
            put(ringB, lambda: nc.scalar.dma_start(out=b2_sb[:], in_=b2_d[:, :]))

            w1_sb = wpool.tile([128, F, NTD], bf16)   # w1_sb[p,f,dt] = w1[f, dt*128+p]
            w2_sb = wpool.tile([128, D, NTF], bf16)   # w2_sb[p,d,ft] = w2[d, ft*128+p]
            FC = F // 8
            for i in range(8):
                ring = ringA if i % 2 == 0 else ringB
                eng = nc.sync if i % 2 == 0 else nc.scalar
                put(ring, lambda i=i, eng=eng: eng.dma_start(
                    out=w1_sb[:, i * FC : (i + 1) * FC, :],
                    in_=w1_d[:, i * FC : (i + 1) * FC, :]))
            DC = D // 8
            for i in range(8):
                ring = ringA if i % 2 == 0 else ringB
                eng = nc.sync if i % 2 == 0 else nc.scalar
                put(ring, lambda i=i, eng=eng: eng.dma_start(
                    out=w2_sb[:, i * DC : (i + 1) * DC, :],
                    in_=w2_d[:, i * DC : (i + 1) * DC, :]))
            for b in range(1, NB):
                ring = ringA if b % 2 == 1 else ringB
                eng = nc.sync if b % 2 == 1 else nc.scalar
                put(ring, lambda b=b, eng=eng: eng.dma_start(
                    out=xg[b][:], in_=xg_d[b][:, :, :]))

            # ---- FFN over token blocks ---------------------------------
            for blk, TB in enumerate(blocks):
                xg_blk = xg[blk]
                h_sb = hpool.tile([128, NTF, max(blocks)], bf16, tag="h")
                for ft in range(NTF):
                    hp = psH.tile([128, TB], f32, tag="hps")
                    for dt in range(NTD):
                        nc.tensor.matmul(
                            hp[:],
                            lhsT=w1_sb[:, ft * 128 : (ft + 1) * 128, dt],
                            rhs=xg_blk[:, dt, :],
                            start=(dt == 0),
                            stop=(dt == NTD - 1),
                        )
                    nc.scalar.activation(out=h_sb[:, ft, 0:TB], in_=hp[:], func=ACT.Relu,
                                         bias=b1_sb[:, ft : ft + 1], scale=1.0)
                y_blk = ypool.tile([128, NTD, TB], bf16, tag="y")
                for dt in range(NTD):
                    yp = psY.tile([128, TB], f32, tag="yps")
                    for ft in range(NTF):
                        nc.tensor.matmul(
                            yp[:],
                            lhsT=w2_sb[:, dt * 128 : (dt + 1) * 128, ft],
                            rhs=h_sb[:, ft, 0:TB],
                            start=(ft == 0),
                            stop=(ft == NTF - 1),
                        )
                    nc.vector.tensor_scalar_add(y_blk[:, dt, :], yp[:], b2_sb[:, dt : dt + 1])
                    if blk == len(blocks) - 1:
                        nc.sync.dma_start(out=yg_d[blk][:, dt, :], in_=y_blk[:, dt, :])
                if blk < len(blocks) - 1:
                    nc.sync.dma_start(out=yg_d[blk][:, :, :], in_=y_blk[:])

    nc.compile()
    return nc


def _get_nc(blocks: tuple):
    if blocks not in _cache:
        _cache[blocks] = _build(blocks)
    return _cache[blocks]


def kernel(x, gate_w, w1, b1, w2, b2, k):
    from concourse.bass_utils import run_bass_kernel_spmd

    assert int(k) == 2
    x = np.asarray(x, dtype=np.float32)
    gate_w = np.asarray(gate_w, dtype=np.float32)
    w1 = np.asarray(w1, dtype=np.float32)
    b1 = np.asarray(b1, dtype=np.float32)
    w2 = np.asarray(w2, dtype=np.float32)
    b2 = np.asarray(b2, dtype=np.float32)
    B, T, _ = x.shape
    xf = x.reshape(S, D)

    # Router (exact fp32, matching the reference's top-2 renormalized
    # softmax; gates applied host-side during the merge).
    logits = xf @ gate_w.T
    top2 = np.argpartition(-logits, 2, axis=1)[:, :2]
    topv = np.take_along_axis(logits, top2, axis=1)              # (S, 2)
    ex = np.exp(topv - topv.max(axis=1, keepdims=True))
    gsm = ex / ex.sum(axis=1, keepdims=True)
    gates = np.zeros((S, E), dtype=np.float32)
    np.put_along_axis(gates, top2, gsm.astype(np.float32), axis=1)

    sel = np.zeros((S, E), dtype=bool)
    np.put_along_axis(sel, top2, True, axis=1)
    toks = [np.nonzero(sel[:, e])[0] for e in range(E)]
    maxcnt = max(len(t) for t in toks)

    blocks = _plan_blocks(maxcnt)
    C = sum(blocks)
    offs = [sum(blocks[:i]) for i in range(len(blocks))]
    nc = _get_nc(blocks)

    xfT16 = np.ascontiguousarray(xf.T).astype(ml_dtypes.bfloat16)  # [D, S]
    in_maps = []
    for c in range(E):
        tp = np.zeros(C, dtype=np.int64)
        tp[: len(toks[c])] = toks[c]
        im = {
            # w1f[p, f, dt] = w1[c][f, dt*128+p]
            "w1f": np.ascontiguousarray(
                w1[c].astype(ml_dtypes.bfloat16).reshape(F, NTD, 128).transpose(2, 0, 1)),
            # w2f[p, d, ft] = w2[c][d, ft*128+p]
            "w2f": np.ascontiguousarray(
                w2[c].astype(ml_dtypes.bfloat16).reshape(D, NTF, 128).transpose(2, 0, 1)),
            "b1r": np.ascontiguousarray(b1[c].reshape(NTF, 128).T),
            "b2r": np.ascontiguousarray(b2[c].reshape(NTD, 128).T),
        }
        for b in range(len(blocks)):
            g = xfT16[:, tp[offs[b] : offs[b] + blocks[b]]]       # [D, TB]
            im[f"xg{b}"] = np.ascontiguousarray(
                g.reshape(NTD, 128, blocks[b]).transpose(1, 0, 2))
        in_maps.append(im)

    res = run_bass_kernel_spmd(nc, in_maps, core_ids=list(range(8)))
    global LAST_RES
    LAST_RES = res

    out = np.zeros((S, D), dtype=np.float32)
    for c in range(E):
        cnt = len(toks[c])
        yt = np.concatenate(
            [np.asarray(res.results[c][f"yg{b}"]).astype(np.float32)
             .transpose(1, 0, 2).reshape(D, blocks[b])
             for b in range(len(blocks))], axis=1)                # [D, C]
        out[toks[c]] += yt[:, :cnt].T * gates[toks[c], c][:, None]
    return out.reshape(B, T, D)


# revision 8
# speedup vs baseline: 1.1074x; 1.1074x over previous
"""MoE positionwise FFN (top-2 of 8 experts) on 8 TRN2 NeuronCores.

Strategy: expert-parallel, host-routed. The router (logits -> top-2 ->
softmax gates) is exact fp32 on host (as is the final scatter-add
combine, matching the reference semantics). Each core owns one expert:
the host gathers that expert's routed tokens into a compact bf16
input, and the device kernel is a pure dense FFN:

    h = relu(w1 @ x + b1)   (D -> F)
    y = w2 @ h + b2         (F -> D)

over C tokens in blocks (first block largest so block-0 compute
consumes the weight stream slower than DMA supplies it), weight-
stationary bf16 matmuls, fp32 PSUM accumulation.

All device inputs are pre-permuted on host into the exact SBUF layout
so every DMA is a contiguous slice (>=8KB per-partition runs, full HBM
bandwidth): w1 as [128, F, NTD] (f-sliced chunks stream while block-0
computes), w2 as [128, D, NTF] (d-sliced), x per block as [128, NTD,
TB]. Weight chunks alternate across both HWDGE rings (sync + scalar).
The matmul lhsT reads these with a small free-dim stride, which
LDWEIGHTS tolerates (stays hidden behind the matmuls). The PE is
pre-warmed so the HAM clock gate is at 8/8 when real matmuls start.

Self-contained: hardcodes shapes for B=2,T=2048,D=1024,F=4096,E=8,K=2.
"""
import math

import numpy as np
import ml_dtypes

S = 4096
D = 1024
F = 4096
E = 8
NTD = D // 128   # 8 d-tiles
NTF = F // 128   # 32 f-tiles

_cache: dict = {}
LAST_RES = None


def _plan_blocks(maxcnt: int):
    C = max(192, int(math.ceil(maxcnt / 16)) * 16)
    s0 = min(448, C)
    rem = C - s0
    if rem == 0:
        return (s0,)
    s1 = int(math.ceil(rem / 2 / 16)) * 16
    s2 = rem - s1
    if s2 == 0:
        return (s0, s1)
    return (s0, s1, s2)


def _build(blocks: tuple):
    import concourse.bacc as bacc
    import concourse.tile as tile
    import concourse.mybir as mybir
    from concourse.tile import add_dep_helper

    f32 = mybir.dt.float32
    bf16 = mybir.dt.bfloat16
    ACT = mybir.ActivationFunctionType

    NB = len(blocks)

    nc = bacc.Bacc("TRN2", target_bir_lowering=False, debug=False, num_devices=8)

    xg_d = [nc.dram_tensor(f"xg{b}", [128, NTD, blocks[b]], bf16,
                           kind="ExternalInput") for b in range(NB)]
    w1_d = nc.dram_tensor("w1f", [128, F, NTD], bf16, kind="ExternalInput")
    w2_d = nc.dram_tensor("w2f", [128, D, NTF], bf16, kind="ExternalInput")
    b1_d = nc.dram_tensor("b1r", [128, NTF], f32, kind="ExternalInput")
    b2_d = nc.dram_tensor("b2r", [128, NTD], f32, kind="ExternalInput")
    yg_d = [nc.dram_tensor(f"yg{b}", [128, NTD, blocks[b]], bf16,
                           kind="ExternalOutput") for b in range(NB)]

    with tile.TileContext(nc) as tc:
        with (
            tc.tile_pool(name="wpool", bufs=1) as wpool,
            tc.tile_pool(name="xr", bufs=1) as xr,
            tc.tile_pool(name="small", bufs=1) as small,
            tc.tile_pool(name="hpool", bufs=1) as hpool,
            tc.tile_pool(name="ypool", bufs=2) as ypool,
            tc.tile_pool(name="psH", bufs=3, space="PSUM") as psH,
            tc.tile_pool(name="psY", bufs=2, space="PSUM") as psY,
            tc.tile_pool(name="psW", bufs=1, space="PSUM") as psW,
        ):
            # ---- PE pre-warm: trip the HAM activity window during the
            # initial DMA so real matmuls start at 2.4 GHz.
            junk = small.tile([128, 256], bf16)
            nc.vector.memset(junk[:], 0.0)
            wps = psW.tile([128, 256], f32)
            for _ in range(12):
                nc.tensor.matmul(wps[:], lhsT=junk[:, 0:128], rhs=junk[:],
                                 start=True, stop=True)

            # ---- DMA streams on the two HWDGE rings.
            # ring B (scalar): xg block0 + biases, then odd weight chunks
            #                  (+ xg block2).
            # ring A (sync):   even weight chunks (+ xg block1); y-out
            #                  rides sync after everything.
            ringA = []  # sync
            ringB = []  # scalar

            def put(ring, fn):
                # depth-2 pipeline per HWDGE ring: issue order via a
                # scheduling edge to the previous DMA, and a completion
                # dep on the one before that. Keeps each ring streaming
                # back-to-back (receipt latency overlapped) while
                # bounding in-flight DMAs to 2/ring (8 DMAHW sem lanes).
                d = fn()
                if ring:
                    add_dep_helper(d.ins, ring[-1].ins, sync=False,
                                   reason="ring order")
                if len(ring) >= 2:
                    add_dep_helper(d.ins, ring[-2].ins, sync=True,
                                   reason="ring depth-2")
                ring.append(d)
                return d

            xg = [xr.tile([128, NTD, blocks[b]], bf16, tag=f"xg{b}",
                          name=f"xg{b}")
                  for b in range(NB)]
            put(ringB, lambda: nc.scalar.dma_start(out=xg[0][:], in_=xg_d[0][:, :, :]))
            b1_sb = small.tile([128, NTF], f32)
            put(ringB, lambda: nc.scalar.dma_start(out=b1_sb[:], in_=b1_d[:, :]))
            b2_sb = small.tile([128, NTD], f32)
            put(ringB, lambda: nc.scalar.dma_start(out=b2_sb[:], in_=b2_d[:, :]))

            w1_sb = wpool.tile([128, F, NTD], bf16)   # w1_sb[p,f,dt] = w1[f, dt*128+p]
            w2_sb = wpool.tile([128, D, NTF], bf16)   # w2_sb[p,d,ft] = w2[d, ft*128+p]
            FC = F // 8
            for i in range(8):
                ring = ringA if i % 2 == 0 else ringB
                eng = nc.sync if i % 2 == 0 else nc.scalar
                put(ring, lambda i=i, eng=eng: eng.dma_start(
                    out=w1_sb[:, i * FC : (i + 1) * FC, :],
                    in_=w1_d[:, i * FC : (i + 1) * FC, :]))
            DC = D // 8
            for i in range(8):
                ring = ringA if i % 2 == 0 else ringB
                eng = nc.sync if i % 2 == 0 else nc.scalar
                put(ring, lambda i=i, eng=eng: eng.dma_start(
                    out=w2_sb[:, i * DC : (i + 1) * DC, :],
                    in_=w2_d[:, i * DC : (i + 1) * DC, :]))
            for b in range(1, NB):
                ring = ringA if b % 2 == 1 else ringB
                eng = nc.sync if b % 2 == 1 else nc.scalar
                put(ring, lambda b=b, eng=eng: eng.dma_start(
                    out=xg[b][:], in_=xg_d[b][:, :, :]))

            # ---- FFN over token blocks ---------------------------------
            for blk, TB in enumerate(blocks):
                xg_blk = xg[blk]
                h_sb = hpool.tile([128, NTF, max(blocks)], bf16, tag="h")
                for ft in range(NTF):
                    hp = psH.tile([128, TB], f32, tag="hps")
                    for dt in range(NTD):
                        nc.tensor.matmul(
                            hp[:],
                            lhsT=w1_sb[:, ft * 128 : (ft + 1) * 128, dt],
                            rhs=xg_blk[:, dt, :],
                            start=(dt == 0),
                            stop=(dt == NTD - 1),
                        )
                    nc.scalar.activation(out=h_sb[:, ft, 0:TB], in_=hp[:], func=ACT.Relu,
                                         bias=b1_sb[:, ft : ft + 1], scale=1.0)
                y_blk = ypool.tile([128, NTD, TB], bf16, tag="y")
                for dt in range(NTD):
                    yp = psY.tile([128, TB], f32, tag="yps")
                    for ft in range(NTF):
                        nc.tensor.matmul(
                            yp[:],
                            lhsT=w2_sb[:, dt * 128 : (dt + 1) * 128, ft],
                            rhs=h_sb[:, ft, 0:TB],
                            start=(ft == 0),
                            stop=(ft == NTF - 1),
                        )
                    nc.vector.tensor_scalar_add(y_blk[:, dt, :], yp[:], b2_sb[:, dt : dt + 1])
                    if blk == len(blocks) - 1:
                        nc.sync.dma_start(out=yg_d[blk][:, dt, :], in_=y_blk[:, dt, :])
                if blk < len(blocks) - 1:
                    nc.sync.dma_start(out=yg_d[blk][:, :, :], in_=y_blk[:])

    nc.compile()
    return nc


def _get_nc(blocks: tuple):
    if blocks not in _cache:
        _cache[blocks] = _build(blocks)
    return _cache[blocks]


def kernel(x, gate_w, w1, b1, w2, b2, k):
    from concourse.bass_utils import run_bass_kernel_spmd

    assert int(k) == 2
    x = np.asarray(x, dtype=np.float32)
    gate_w = np.asarray(gate_w, dtype=np.float32)
    w1 = np.asarray(w1, dtype=np.float32)
    b1 = np.asarray(b1, dtype=np.float32)
    w2 = np.asarray(w2, dtype=np.float32)
    b2 = np.asarray(b2, dtype=np.float32)
    B, T, _ = x.shape
    xf = x.reshape(S, D)

    # Router (exact fp32, matching the reference's top-2 renormalized
    # softmax; gates applied host-side during the merge).
    logits = xf @ gate_w.T
    top2 = np.argpartition(-logits, 2, axis=1)[:, :2]
    topv = np.take_along_axis(logits, top2, axis=1)              # (S, 2)
    ex = np.exp(topv - topv.max(axis=1, keepdims=True))
    gsm = ex / ex.sum(axis=1, keepdims=True)
    gates = np.zeros((S, E), dtype=np.float32)
    np.put_along_axis(gates, top2, gsm.astype(np.float32), axis=1)

    sel = np.zeros((S, E), dtype=bool)
    np.put_along_axis(sel, top2, True, axis=1)
    toks = [np.nonzero(sel[:, e])[0] for e in range(E)]
    maxcnt = max(len(t) for t in toks)

    blocks = _plan_blocks(maxcnt)
    C = sum(blocks)
    offs = [sum(blocks[:i]) for i in range(len(blocks))]
    nc = _get_nc(blocks)

    xfT16 = np.ascontiguousarray(xf.T).astype(ml_dtypes.bfloat16)  # [D, S]
    in_maps = []
    for c in range(E):
        tp = np.zeros(C, dtype=np.int64)
        tp[: len(toks[c])] = toks[c]
        im = {
            # w1f[p, f, dt] = w1[c][f, dt*128+p]
            "w1f": np.ascontiguousarray(
                w1[c].astype(ml_dtypes.bfloat16).reshape(F, NTD, 128).transpose(2, 0, 1)),
            # w2f[p, d, ft] = w2[c][d, ft*128+p]
            "w2f": np.ascontiguousarray(
                w2[c].astype(ml_dtypes.bfloat16).reshape(D, NTF, 128).transpose(2, 0, 1)),
            "b1r": np.ascontiguousarray(b1[c].reshape(NTF, 128).T),
            "b2r": np.ascontiguousarray(b2[c].reshape(NTD, 128).T),
        }
        for b in range(len(blocks)):
            g = xfT16[:, tp[offs[b] : offs[b] + blocks[b]]]       # [D, TB]
            im[f"xg{b}"] = np.ascontiguousarray(
                g.reshape(NTD, 128, blocks[b]).transpose(1, 0, 2))
        in_maps.append(im)

    res = run_bass_kernel_spmd(nc, in_maps, core_ids=list(range(8)))
    global LAST_RES
    LAST_RES = res

    out = np.zeros((S, D), dtype=np.float32)
    for c in range(E):
        cnt = len(toks[c])
        yt = np.concatenate(
            [np.asarray(res.results[c][f"yg{b}"]).astype(np.float32)
             .transpose(1, 0, 2).reshape(D, blocks[b])
             for b in range(len(blocks))], axis=1)                # [D, C]
        out[toks[c]] += yt[:, :cnt].T * gates[toks[c], c][:, None]
    return out.reshape(B, T, D)


# revision 10
# speedup vs baseline: 1.2820x; 1.1577x over previous
"""MoE positionwise FFN (top-2 of 8 experts) on 8 TRN2 NeuronCores.

Strategy: expert-parallel, host-routed. The router (logits -> top-2 ->
softmax gates) is exact fp32 on host (as is the final scatter-add
combine, matching the reference semantics). Each core owns one expert:
the host gathers that expert's routed tokens into a compact bf16
input, and the device kernel is a pure dense FFN:

    h = relu(w1 @ x + b1)   (D -> F)
    y = w2 @ h + b2         (F -> D)

over C tokens in blocks (first block largest so block-0 compute
consumes the weight stream slower than DMA supplies it), weight-
stationary bf16 matmuls, fp32 PSUM accumulation.

All device inputs are pre-permuted on host into the exact SBUF layout
so every DMA is a contiguous slice (>=8KB per-partition runs, full HBM
bandwidth): w1 as [128, F, NTD] (f-sliced chunks stream while block-0
computes), w2 as [128, D, NTF] (d-sliced), x per block as [128, NTD,
TB]. Weight chunks alternate across both HWDGE rings (sync + scalar).
The matmul lhsT reads these with a small free-dim stride, which
LDWEIGHTS tolerates (stays hidden behind the matmuls). The PE is
pre-warmed so the HAM clock gate is at 8/8 when real matmuls start.

Self-contained: hardcodes shapes for B=2,T=2048,D=1024,F=4096,E=8,K=2.
"""
import math

import numpy as np
import ml_dtypes

S = 4096
D = 1024
F = 4096
E = 8
NTD = D // 128   # 8 d-tiles
NTF = F // 128   # 32 f-tiles

_cache: dict = {}
LAST_RES = None


def _plan_blocks(maxcnt: int):
    C = max(192, int(math.ceil(maxcnt / 16)) * 16)
    s0 = min(448, C)
    rem = C - s0
    if rem == 0:
        return (s0,)
    s1 = int(math.ceil(rem / 2 / 16)) * 16
    s2 = rem - s1
    if s2 == 0:
        return (s0, s1)
    return (s0, s1, s2)


def _build(blocks: tuple):
    import concourse.bacc as bacc
    import concourse.tile as tile
    import concourse.mybir as mybir
    from concourse.tile import add_dep_helper

    f32 = mybir.dt.float32
    bf16 = mybir.dt.bfloat16
    ACT = mybir.ActivationFunctionType

    NB = len(blocks)

    nc = bacc.Bacc("TRN2", target_bir_lowering=False, debug=False, num_devices=8)

    xg_d = [nc.dram_tensor(f"xg{b}", [128, NTD, blocks[b]], bf16,
                           kind="ExternalInput") for b in range(NB)]
    w1_d = nc.dram_tensor("w1f", [128, F, NTD], bf16, kind="ExternalInput")
    w2_d = nc.dram_tensor("w2f", [128, D, NTF], bf16, kind="ExternalInput")
    b1_d = nc.dram_tensor("b1r", [128, NTF], f32, kind="ExternalInput")
    b2_d = nc.dram_tensor("b2r", [128, NTD], f32, kind="ExternalInput")
    yg_d = [nc.dram_tensor(f"yg{b}", [128, NTD, blocks[b]], bf16,
                           kind="ExternalOutput") for b in range(NB)]

    with tile.TileContext(nc) as tc:
        with (
            tc.tile_pool(name="wpool", bufs=1) as wpool,
            tc.tile_pool(name="xr", bufs=1) as xr,
            tc.tile_pool(name="small", bufs=1) as small,
            tc.tile_pool(name="hpool", bufs=1) as hpool,
            tc.tile_pool(name="ypool", bufs=2) as ypool,
            tc.tile_pool(name="psH", bufs=3, space="PSUM") as psH,
            tc.tile_pool(name="psY", bufs=2, space="PSUM") as psY,
            tc.tile_pool(name="psW", bufs=1, space="PSUM") as psW,
        ):
            # ---- PE pre-warm: trip the HAM activity window during the
            # initial DMA so real matmuls start at 2.4 GHz.
            junk = small.tile([128, 256], bf16)
            nc.vector.memset(junk[:], 0.0)
            wps = psW.tile([128, 256], f32)
            for _ in range(16):
                nc.tensor.matmul(wps[:], lhsT=junk[:, 0:128], rhs=junk[:],
                                 start=True, stop=True)

            # ---- DMA streams. Everything big rides the SP (sync) HWDGE
            # ring, depth-2 pipelined: each DMA gets a scheduling edge to
            # the previous one (issue order) and a completion dep on the
            # one before that, so the ring streams back-to-back (receipt
            # latency overlapped) with at most 2 in flight. The ACT
            # (scalar) engine carries ONLY the two dep-free bias DMAs —
            # stream DMAs there would head-of-line-block the activations
            # behind their completion waits, stalling PSUM drain and the
            # PE with it.
            ring = []

            def put(fn):
                d = fn()
                if ring:
                    add_dep_helper(d.ins, ring[-1].ins, sync=False,
                                   reason="ring order")
                if len(ring) >= 2:
                    add_dep_helper(d.ins, ring[-2].ins, sync=True,
                                   reason="ring depth-2")
                ring.append(d)
                return d

            b1_sb = small.tile([128, NTF], f32)
            nc.scalar.dma_start(out=b1_sb[:], in_=b1_d[:, :])
            b2_sb = small.tile([128, NTD], f32)
            nc.scalar.dma_start(out=b2_sb[:], in_=b2_d[:, :])

            xg = [xr.tile([128, NTD, blocks[b]], bf16, tag=f"xg{b}",
                          name=f"xg{b}")
                  for b in range(NB)]
            put(lambda: nc.sync.dma_start(out=xg[0][:], in_=xg_d[0][:, :, :]))

            w1_sb = wpool.tile([128, F, NTD], bf16)   # w1_sb[p,f,dt] = w1[f, dt*128+p]
            w2_sb = wpool.tile([128, D, NTF], bf16)   # w2_sb[p,d,ft] = w2[d, ft*128+p]
            # fine-grained first chunks so block-0 ft=0 can start early
            w1cuts = [0, 128, 256, 512, 1024, 1536, 2048, 2560, 3072, 3584, F]
            for lo, hi in zip(w1cuts, w1cuts[1:]):
                put(lambda lo=lo, hi=hi: nc.sync.dma_start(
                    out=w1_sb[:, lo:hi, :], in_=w1_d[:, lo:hi, :]))
            DC = D // 8
            for i in range(8):
                put(lambda i=i: nc.sync.dma_start(
                    out=w2_sb[:, i * DC : (i + 1) * DC, :],
                    in_=w2_d[:, i * DC : (i + 1) * DC, :]))
            for b in range(1, NB):
                put(lambda b=b: nc.sync.dma_start(
                    out=xg[b][:], in_=xg_d[b][:, :, :]))

            # ---- FFN over token blocks ---------------------------------
            for blk, TB in enumerate(blocks):
                xg_blk = xg[blk]
                h_sb = hpool.tile([128, NTF, max(blocks)], bf16, tag="h")
                for ft in range(NTF):
                    hp = psH.tile([128, TB], f32, tag="hps")
                    for dt in range(NTD):
                        nc.tensor.matmul(
                            hp[:],
                            lhsT=w1_sb[:, ft * 128 : (ft + 1) * 128, dt],
                            rhs=xg_blk[:, dt, :],
                            start=(dt == 0),
                            stop=(dt == NTD - 1),
                        )
                    nc.scalar.activation(out=h_sb[:, ft, 0:TB], in_=hp[:], func=ACT.Relu,
                                         bias=b1_sb[:, ft : ft + 1], scale=1.0)
                y_blk = ypool.tile([128, NTD, TB], bf16, tag="y")
                for dt in range(NTD):
                    yp = psY.tile([128, TB], f32, tag="yps")
                    for ft in range(NTF):
                        nc.tensor.matmul(
                            yp[:],
                            lhsT=w2_sb[:, dt * 128 : (dt + 1) * 128, ft],
                            rhs=h_sb[:, ft, 0:TB],
                            start=(ft == 0),
                            stop=(ft == NTF - 1),
                        )
                    nc.vector.tensor_scalar_add(y_blk[:, dt, :], yp[:], b2_sb[:, dt : dt + 1])
                    if blk == len(blocks) - 1:
                        put(lambda blk=blk, dt=dt, y_blk=y_blk: nc.sync.dma_start(
                            out=yg_d[blk][:, dt, :], in_=y_blk[:, dt, :]))
                if blk < len(blocks) - 1:
                    put(lambda blk=blk, y_blk=y_blk: nc.sync.dma_start(
                        out=yg_d[blk][:, :, :], in_=y_blk[:]))

    nc.compile()
    return nc


def _get_nc(blocks: tuple):
    if blocks not in _cache:
        _cache[blocks] = _build(blocks)
    return _cache[blocks]


def kernel(x, gate_w, w1, b1, w2, b2, k):
    from concourse.bass_utils import run_bass_kernel_spmd

    assert int(k) == 2
    x = np.asarray(x, dtype=np.float32)
    gate_w = np.asarray(gate_w, dtype=np.float32)
    w1 = np.asarray(w1, dtype=np.float32)
    b1 = np.asarray(b1, dtype=np.float32)
    w2 = np.asarray(w2, dtype=np.float32)
    b2 = np.asarray(b2, dtype=np.float32)
    B, T, _ = x.shape
    xf = x.reshape(S, D)

    # Router (exact fp32, matching the reference's top-2 renormalized
    # softmax; gates applied host-side during the merge).
    logits = xf @ gate_w.T
    top2 = np.argpartition(-logits, 2, axis=1)[:, :2]
    topv = np.take_along_axis(logits, top2, axis=1)              # (S, 2)
    ex = np.exp(topv - topv.max(axis=1, keepdims=True))
    gsm = ex / ex.sum(axis=1, keepdims=True)
    gates = np.zeros((S, E), dtype=np.float32)
    np.put_along_axis(gates, top2, gsm.astype(np.float32), axis=1)

    sel = np.zeros((S, E), dtype=bool)
    np.put_along_axis(sel, top2, True, axis=1)
    toks = [np.nonzero(sel[:, e])[0] for e in range(E)]
    maxcnt = max(len(t) for t in toks)

    blocks = _plan_blocks(maxcnt)
    C = sum(blocks)
    offs = [sum(blocks[:i]) for i in range(len(blocks))]
    nc = _get_nc(blocks)

    xfT16 = np.ascontiguousarray(xf.T).astype(ml_dtypes.bfloat16)  # [D, S]
    in_maps = []
    for c in range(E):
        tp = np.zeros(C, dtype=np.int64)
        tp[: len(toks[c])] = toks[c]
        im = {
            # w1f[p, f, dt] = w1[c][f, dt*128+p]
            "w1f": np.ascontiguousarray(
                w1[c].astype(ml_dtypes.bfloat16).reshape(F, NTD, 128).transpose(2, 0, 1)),
            # w2f[p, d, ft] = w2[c][d, ft*128+p]
            "w2f": np.ascontiguousarray(
                w2[c].astype(ml_dtypes.bfloat16).reshape(D, NTF, 128).transpose(2, 0, 1)),
            "b1r": np.ascontiguousarray(b1[c].reshape(NTF, 128).T),
            "b2r": np.ascontiguousarray(b2[c].reshape(NTD, 128).T),
        }
        for b in range(len(blocks)):
            g = xfT16[:, tp[offs[b] : offs[b] + blocks[b]]]       # [D, TB]
            im[f"xg{b}"] = np.ascontiguousarray(
                g.reshape(NTD, 128, blocks[b]).transpose(1, 0, 2))
        in_maps.append(im)

    res = run_bass_kernel_spmd(nc, in_maps, core_ids=list(range(8)))
    global LAST_RES
    LAST_RES = res

    out = np.zeros((S, D), dtype=np.float32)
    for c in range(E):
        cnt = len(toks[c])
        yt = np.concatenate(
            [np.asarray(res.results[c][f"yg{b}"]).astype(np.float32)
             .transpose(1, 0, 2).reshape(D, blocks[b])
             for b in range(len(blocks))], axis=1)                # [D, C]
        out[toks[c]] += yt[:, :cnt].T * gates[toks[c], c][:, None]
    return out.reshape(B, T, D)


# revision 11
# speedup vs baseline: 1.3082x; 1.0204x over previous
"""MoE positionwise FFN (top-2 of 8 experts) on 8 TRN2 NeuronCores.

Strategy: expert-parallel, host-routed. The router (logits -> top-2 ->
softmax gates) is exact fp32 on host (as is the final scatter-add
combine, matching the reference semantics). Each core owns one expert:
the host gathers that expert's routed tokens into a compact bf16
input, and the device kernel is a pure dense FFN:

    h = relu(w1 @ x + b1)   (D -> F)
    y = w2 @ h + b2         (F -> D)

over C tokens in blocks (first block largest so block-0 compute
consumes the weight stream slower than DMA supplies it), weight-
stationary bf16 matmuls, fp32 PSUM accumulation.

All device inputs are pre-permuted on host into the exact SBUF layout
so every DMA is a contiguous slice (>=8KB per-partition runs, full HBM
bandwidth): w1 as [128, F, NTD] (f-sliced chunks stream while block-0
computes), w2 as [128, D, NTF] (d-sliced), x per block as [128, NTD,
TB]. Weight chunks alternate across both HWDGE rings (sync + scalar).
The matmul lhsT reads these with a small free-dim stride, which
LDWEIGHTS tolerates (stays hidden behind the matmuls). The PE is
pre-warmed so the HAM clock gate is at 8/8 when real matmuls start.

Self-contained: hardcodes shapes for B=2,T=2048,D=1024,F=4096,E=8,K=2.
"""
import math

import numpy as np
import ml_dtypes

S = 4096
D = 1024
F = 4096
E = 8
NTD = D // 128   # 8 d-tiles
NTF = F // 128   # 32 f-tiles

_cache: dict = {}
LAST_RES = None


def _plan_blocks(maxcnt: int):
    C = max(192, int(math.ceil(maxcnt / 16)) * 16)
    s0 = min(448, C)
    rem = C - s0
    if rem == 0:
        return (s0,)
    s1 = int(math.ceil(rem / 2 / 16)) * 16
    s2 = rem - s1
    if s2 == 0:
        return (s0, s1)
    return (s0, s1, s2)


def _build(blocks: tuple):
    import concourse.bacc as bacc
    import concourse.tile as tile
    import concourse.mybir as mybir
    from concourse.tile import add_dep_helper

    f32 = mybir.dt.float32
    bf16 = mybir.dt.bfloat16
    ACT = mybir.ActivationFunctionType

    NB = len(blocks)

    nc = bacc.Bacc("TRN2", target_bir_lowering=False, debug=False, num_devices=8)

    xg_d = [nc.dram_tensor(f"xg{b}", [128, NTD, blocks[b]], bf16,
                           kind="ExternalInput") for b in range(NB)]
    w1_d = nc.dram_tensor("w1f", [128, F, NTD], bf16, kind="ExternalInput")
    w2_d = nc.dram_tensor("w2f", [128, D, NTF], bf16, kind="ExternalInput")
    b1_d = nc.dram_tensor("b1r", [128, NTF], f32, kind="ExternalInput")
    b2_d = nc.dram_tensor("b2r", [128, NTD], f32, kind="ExternalInput")
    yg_d = [nc.dram_tensor(f"yg{b}", [128, NTD, blocks[b]], bf16,
                           kind="ExternalOutput") for b in range(NB)]

    with tile.TileContext(nc) as tc:
        with (
            tc.tile_pool(name="wpool", bufs=1) as wpool,
            tc.tile_pool(name="xr", bufs=1) as xr,
            tc.tile_pool(name="small", bufs=1) as small,
            tc.tile_pool(name="hpool", bufs=1) as hpool,
            tc.tile_pool(name="ypool", bufs=2) as ypool,
            tc.tile_pool(name="psH", bufs=3, space="PSUM") as psH,
            tc.tile_pool(name="psY", bufs=2, space="PSUM") as psY,
            tc.tile_pool(name="psW", bufs=1, space="PSUM") as psW,
        ):
            # ---- PE pre-warm: trip the HAM activity window during the
            # initial DMA so real matmuls start at 2.4 GHz.
            junk = small.tile([128, 256], bf16)
            nc.vector.memset(junk[:], 0.0)
            wps = psW.tile([128, 256], f32)
            for _ in range(20):
                nc.tensor.matmul(wps[:], lhsT=junk[:, 0:128], rhs=junk[:],
                                 start=True, stop=True)

            # ---- DMA streams. Everything big rides the SP (sync) HWDGE
            # ring, depth-2 pipelined: each DMA gets a scheduling edge to
            # the previous one (issue order) and a completion dep on the
            # one before that, so the ring streams back-to-back (receipt
            # latency overlapped) with at most 2 in flight. The ACT
            # (scalar) engine carries ONLY the two dep-free bias DMAs —
            # stream DMAs there would head-of-line-block the activations
            # behind their completion waits, stalling PSUM drain and the
            # PE with it.
            ring = []

            def put(fn):
                d = fn()
                if ring:
                    add_dep_helper(d.ins, ring[-1].ins, sync=False,
                                   reason="ring order")
                if len(ring) >= 3:
                    add_dep_helper(d.ins, ring[-3].ins, sync=True,
                                   reason="ring depth-3")
                ring.append(d)
                return d

            b1_sb = small.tile([128, NTF], f32)
            nc.scalar.dma_start(out=b1_sb[:], in_=b1_d[:, :])
            b2_sb = small.tile([128, NTD], f32)
            nc.scalar.dma_start(out=b2_sb[:], in_=b2_d[:, :])

            xg = [xr.tile([128, NTD, blocks[b]], bf16, tag=f"xg{b}",
                          name=f"xg{b}")
                  for b in range(NB)]
            put(lambda: nc.sync.dma_start(out=xg[0][:], in_=xg_d[0][:, :, :]))

            w1_sb = wpool.tile([128, F, NTD], bf16)   # w1_sb[p,f,dt] = w1[f, dt*128+p]
            w2_sb = wpool.tile([128, D, NTF], bf16)   # w2_sb[p,d,ft] = w2[d, ft*128+p]
            # fine-grained first chunks so block-0 ft=0 can start early
            w1cuts = [0, 128, 256, 512, 1024, 1536, 2048, 2560, 3072, 3584, F]
            for lo, hi in zip(w1cuts, w1cuts[1:]):
                put(lambda lo=lo, hi=hi: nc.sync.dma_start(
                    out=w1_sb[:, lo:hi, :], in_=w1_d[:, lo:hi, :]))
            DC = D // 8
            for i in range(8):
                put(lambda i=i: nc.sync.dma_start(
                    out=w2_sb[:, i * DC : (i + 1) * DC, :],
                    in_=w2_d[:, i * DC : (i + 1) * DC, :]))
            for b in range(1, NB):
                put(lambda b=b: nc.sync.dma_start(
                    out=xg[b][:], in_=xg_d[b][:, :, :]))

            # ---- FFN over token blocks ---------------------------------
            for blk, TB in enumerate(blocks):
                xg_blk = xg[blk]
                h_sb = hpool.tile([128, NTF, max(blocks)], bf16, tag="h")
                for ft in range(NTF):
                    hp = psH.tile([128, TB], f32, tag="hps")
                    for dt in range(NTD):
                        nc.tensor.matmul(
                            hp[:],
                            lhsT=w1_sb[:, ft * 128 : (ft + 1) * 128, dt],
                            rhs=xg_blk[:, dt, :],
                            start=(dt == 0),
                            stop=(dt == NTD - 1),
                        )
                    nc.scalar.activation(out=h_sb[:, ft, 0:TB], in_=hp[:], func=ACT.Relu,
                                         bias=b1_sb[:, ft : ft + 1], scale=1.0)
                y_blk = ypool.tile([128, NTD, TB], bf16, tag="y")
                for dt in range(NTD):
                    yp = psY.tile([128, TB], f32, tag="yps")
                    for ft in range(NTF):
                        nc.tensor.matmul(
                            yp[:],
                            lhsT=w2_sb[:, dt * 128 : (dt + 1) * 128, ft],
                            rhs=h_sb[:, ft, 0:TB],
                            start=(ft == 0),
                            stop=(ft == NTF - 1),
                        )
                    nc.vector.tensor_scalar_add(y_blk[:, dt, :], yp[:], b2_sb[:, dt : dt + 1])
                    if blk == len(blocks) - 1:
                        put(lambda blk=blk, dt=dt, y_blk=y_blk: nc.sync.dma_start(
                            out=yg_d[blk][:, dt, :], in_=y_blk[:, dt, :]))
                if blk < len(blocks) - 1:
                    put(lambda blk=blk, y_blk=y_blk: nc.sync.dma_start(
                        out=yg_d[blk][:, :, :], in_=y_blk[:]))

    nc.compile()
    return nc


def _get_nc(blocks: tuple):
    if blocks not in _cache:
        _cache[blocks] = _build(blocks)
    return _cache[blocks]


def kernel(x, gate_w, w1, b1, w2, b2, k):
    from concourse.bass_utils import run_bass_kernel_spmd

    assert int(k) == 2
    x = np.asarray(x, dtype=np.float32)
    gate_w = np.asarray(gate_w, dtype=np.float32)
    w1 = np.asarray(w1, dtype=np.float32)
    b1 = np.asarray(b1, dtype=np.float32)
    w2 = np.asarray(w2, dtype=np.float32)
    b2 = np.asarray(b2, dtype=np.float32)
    B, T, _ = x.shape
    xf = x.reshape(S, D)

    # Router (exact fp32, matching the reference's top-2 renormalized
    # softmax; gates applied host-side during the merge).
    logits = xf @ gate_w.T
    top2 = np.argpartition(-logits, 2, axis=1)[:, :2]
    topv = np.take_along_axis(logits, top2, axis=1)              # (S, 2)
    ex = np.exp(topv - topv.max(axis=1, keepdims=True))
    gsm = ex / ex.sum(axis=1, keepdims=True)
    gates = np.zeros((S, E), dtype=np.float32)
    np.put_along_axis(gates, top2, gsm.astype(np.float32), axis=1)

    sel = np.zeros((S, E), dtype=bool)
    np.put_along_axis(sel, top2, True, axis=1)
    toks = [np.nonzero(sel[:, e])[0] for e in range(E)]
    maxcnt = max(len(t) for t in toks)

    blocks = _plan_blocks(maxcnt)
    C = sum(blocks)
    offs = [sum(blocks[:i]) for i in range(len(blocks))]
    nc = _get_nc(blocks)

    xfT16 = np.ascontiguousarray(xf.T).astype(ml_dtypes.bfloat16)  # [D, S]
    in_maps = []
    for c in range(E):
        tp = np.zeros(C, dtype=np.int64)
        tp[: len(toks[c])] = toks[c]
        im = {
            # w1f[p, f, dt] = w1[c][f, dt*128+p]
            "w1f": np.ascontiguousarray(
                w1[c].astype(ml_dtypes.bfloat16).reshape(F, NTD, 128).transpose(2, 0, 1)),
            # w2f[p, d, ft] = w2[c][d, ft*128+p]
            "w2f": np.ascontiguousarray(
                w2[c].astype(ml_dtypes.bfloat16).reshape(D, NTF, 128).transpose(2, 0, 1)),
            "b1r": np.ascontiguousarray(b1[c].reshape(NTF, 128).T),
            "b2r": np.ascontiguousarray(b2[c].reshape(NTD, 128).T),
        }
        for b in range(len(blocks)):
            g = xfT16[:, tp[offs[b] : offs[b] + blocks[b]]]       # [D, TB]
            im[f"xg{b}"] = np.ascontiguousarray(
                g.reshape(NTD, 128, blocks[b]).transpose(1, 0, 2))
        in_maps.append(im)

    res = run_bass_kernel_spmd(nc, in_maps, core_ids=list(range(8)))
    global LAST_RES
    LAST_RES = res

    out = np.zeros((S, D), dtype=np.float32)
    for c in range(E):
        cnt = len(toks[c])
        yt = np.concatenate(
            [np.asarray(res.results[c][f"yg{b}"]).astype(np.float32)
             .transpose(1, 0, 2).reshape(D, blocks[b])
             for b in range(len(blocks))], axis=1)                # [D, C]
        out[toks[c]] += yt[:, :cnt].T * gates[toks[c], c][:, None]
    return out.reshape(B, T, D)


# revision 13
# speedup vs baseline: 1.3092x; 1.0008x over previous
"""MoE positionwise FFN (top-2 of 8 experts) on 8 TRN2 NeuronCores.

Strategy: expert-parallel, host-routed. The router (logits -> top-2 ->
softmax gates) is exact fp32 on host (as is the final scatter-add
combine, matching the reference semantics). Each core owns one expert:
the host gathers that expert's routed tokens into a compact bf16
input, and the device kernel is a pure dense FFN:

    h = relu(w1 @ x + b1)   (D -> F)
    y = w2 @ h + b2         (F -> D)

over C tokens in blocks (first block largest so block-0 compute
consumes the weight stream slower than DMA supplies it), weight-
stationary bf16 matmuls, fp32 PSUM accumulation.

All device inputs are pre-permuted on host into the exact SBUF layout
so every DMA is a contiguous slice (>=8KB per-partition runs, full HBM
bandwidth): w1 as [128, F, NTD] (f-sliced chunks stream while block-0
computes), w2 as [128, D, NTF] (d-sliced), x per block as [128, NTD,
TB]. Weight chunks alternate across both HWDGE rings (sync + scalar).
The matmul lhsT reads these with a small free-dim stride, which
LDWEIGHTS tolerates (stays hidden behind the matmuls). The PE is
pre-warmed so the HAM clock gate is at 8/8 when real matmuls start.

Self-contained: hardcodes shapes for B=2,T=2048,D=1024,F=4096,E=8,K=2.
"""
import math

import numpy as np
import ml_dtypes

S = 4096
D = 1024
F = 4096
E = 8
NTD = D // 128   # 8 d-tiles
NTF = F // 128   # 32 f-tiles

_cache: dict = {}
LAST_RES = None


def _plan_blocks(maxcnt: int):
    C = max(192, int(math.ceil(maxcnt / 16)) * 16)
    blocks = []
    rem = C
    while rem > 896:
        blocks.append(448)
        rem -= 448
    if rem > 448:
        s1 = int(math.ceil(rem / 2 / 16)) * 16
        blocks.extend([s1, rem - s1])
    elif rem > 0:
        blocks.append(rem)
    return tuple(blocks)


def _build(blocks: tuple):
    import concourse.bacc as bacc
    import concourse.tile as tile
    import concourse.mybir as mybir
    from concourse.tile import add_dep_helper

    f32 = mybir.dt.float32
    bf16 = mybir.dt.bfloat16
    ACT = mybir.ActivationFunctionType

    NB = len(blocks)

    nc = bacc.Bacc("TRN2", target_bir_lowering=False, debug=False, num_devices=8)

    xg_d = [nc.dram_tensor(f"xg{b}", [128, NTD, blocks[b]], bf16,
                           kind="ExternalInput") for b in range(NB)]
    w1_d = nc.dram_tensor("w1f", [128, F, NTD], bf16, kind="ExternalInput")
    w2_d = nc.dram_tensor("w2f", [128, D, NTF], bf16, kind="ExternalInput")
    b1_d = nc.dram_tensor("b1r", [128, NTF], f32, kind="ExternalInput")
    b2_d = nc.dram_tensor("b2r", [128, NTD], f32, kind="ExternalInput")
    yg_d = [nc.dram_tensor(f"yg{b}", [128, NTD, blocks[b]], bf16,
                           kind="ExternalOutput") for b in range(NB)]

    with tile.TileContext(nc) as tc:
        with (
            tc.tile_pool(name="wpool", bufs=1) as wpool,
            tc.tile_pool(name="xr", bufs=1) as xr,
            tc.tile_pool(name="small", bufs=1) as small,
            tc.tile_pool(name="hpool", bufs=1) as hpool,
            tc.tile_pool(name="ypool", bufs=2) as ypool,
            tc.tile_pool(name="psH", bufs=3, space="PSUM") as psH,
            tc.tile_pool(name="psY", bufs=2, space="PSUM") as psY,
            tc.tile_pool(name="psW", bufs=1, space="PSUM") as psW,
        ):
            # ---- PE pre-warm: trip the HAM activity window during the
            # initial DMA so real matmuls start at 2.4 GHz.
            junk = small.tile([128, 256], bf16)
            nc.vector.memset(junk[:], 0.0)
            wps = psW.tile([128, 256], f32)
            for _ in range(28):
                nc.tensor.matmul(wps[:], lhsT=junk[:, 0:128], rhs=junk[:],
                                 start=True, stop=True)

            # ---- DMA streams. Everything big rides the SP (sync) HWDGE
            # ring, depth-2 pipelined: each DMA gets a scheduling edge to
            # the previous one (issue order) and a completion dep on the
            # one before that, so the ring streams back-to-back (receipt
            # latency overlapped) with at most 2 in flight. The ACT
            # (scalar) engine carries ONLY the two dep-free bias DMAs —
            # stream DMAs there would head-of-line-block the activations
            # behind their completion waits, stalling PSUM drain and the
            # PE with it.
            ring = []

            def put(fn):
                d = fn()
                if ring:
                    add_dep_helper(d.ins, ring[-1].ins, sync=False,
                                   reason="ring order")
                if len(ring) >= 3:
                    add_dep_helper(d.ins, ring[-3].ins, sync=True,
                                   reason="ring depth-3")
                ring.append(d)
                return d

            b1_sb = small.tile([128, NTF], f32)
            nc.scalar.dma_start(out=b1_sb[:], in_=b1_d[:, :])
            b2_sb = small.tile([128, NTD], f32)
            nc.scalar.dma_start(out=b2_sb[:], in_=b2_d[:, :])

            xg = [xr.tile([128, NTD, blocks[b]], bf16, tag=f"xg{b}",
                          name=f"xg{b}")
                  for b in range(NB)]
            put(lambda: nc.sync.dma_start(out=xg[0][:], in_=xg_d[0][:, :, :]))

            w1_sb = wpool.tile([128, F, NTD], bf16)   # w1_sb[p,f,dt] = w1[f, dt*128+p]
            w2_sb = wpool.tile([128, D, NTF], bf16)   # w2_sb[p,d,ft] = w2[d, ft*128+p]
            # fine-grained first chunks so block-0 ft=0 can start early
            w1cuts = [0, 128, 256, 512, 1024, 1536, 2048, 2560, 3072, 3584, F]
            for lo, hi in zip(w1cuts, w1cuts[1:]):
                put(lambda lo=lo, hi=hi: nc.sync.dma_start(
                    out=w1_sb[:, lo:hi, :], in_=w1_d[:, lo:hi, :]))
            DC = D // 8
            for i in range(8):
                put(lambda i=i: nc.sync.dma_start(
                    out=w2_sb[:, i * DC : (i + 1) * DC, :],
                    in_=w2_d[:, i * DC : (i + 1) * DC, :]))
            for b in range(1, NB):
                put(lambda b=b: nc.sync.dma_start(
                    out=xg[b][:], in_=xg_d[b][:, :, :]))

            # ---- FFN over token blocks ---------------------------------
            for blk, TB in enumerate(blocks):
                xg_blk = xg[blk]
                h_sb = hpool.tile([128, NTF, max(blocks)], bf16, tag="h")
                for ft in range(NTF):
                    hp = psH.tile([128, TB], f32, tag="hps")
                    for dt in range(NTD):
                        nc.tensor.matmul(
                            hp[:],
                            lhsT=w1_sb[:, ft * 128 : (ft + 1) * 128, dt],
                            rhs=xg_blk[:, dt, :],
                            start=(dt == 0),
                            stop=(dt == NTD - 1),
                        )
                    nc.scalar.activation(out=h_sb[:, ft, 0:TB], in_=hp[:], func=ACT.Relu,
                                         bias=b1_sb[:, ft : ft + 1], scale=1.0)
                y_blk = ypool.tile([128, NTD, TB], bf16, tag="y")
                for dt in range(NTD):
                    yp = psY.tile([128, TB], f32, tag="yps")
                    for ft in range(NTF):
                        nc.tensor.matmul(
                            yp[:],
                            lhsT=w2_sb[:, dt * 128 : (dt + 1) * 128, ft],
                            rhs=h_sb[:, ft, 0:TB],
                            start=(ft == 0),
                            stop=(ft == NTF - 1),
                        )
                    nc.vector.tensor_scalar_add(y_blk[:, dt, :], yp[:], b2_sb[:, dt : dt + 1])
                    if blk == len(blocks) - 1:
                        put(lambda blk=blk, dt=dt, y_blk=y_blk: nc.sync.dma_start(
                            out=yg_d[blk][:, dt, :], in_=y_blk[:, dt, :]))
                if blk < len(blocks) - 1:
                    put(lambda blk=blk, y_blk=y_blk: nc.sync.dma_start(
                        out=yg_d[blk][:, :, :], in_=y_blk[:]))

    nc.compile()
    return nc


def _get_nc(blocks: tuple):
    if blocks not in _cache:
        _cache[blocks] = _build(blocks)
    return _cache[blocks]


def kernel(x, gate_w, w1, b1, w2, b2, k):
    from concourse.bass_utils import run_bass_kernel_spmd

    assert int(k) == 2
    x = np.asarray(x, dtype=np.float32)
    gate_w = np.asarray(gate_w, dtype=np.float32)
    w1 = np.asarray(w1, dtype=np.float32)
    b1 = np.asarray(b1, dtype=np.float32)
    w2 = np.asarray(w2, dtype=np.float32)
    b2 = np.asarray(b2, dtype=np.float32)
    B, T, _ = x.shape
    xf = x.reshape(S, D)

    # Router (exact fp32, matching the reference's top-2 renormalized
    # softmax; gates applied host-side during the merge).
    logits = xf @ gate_w.T
    top2 = np.argpartition(-logits, 2, axis=1)[:, :2]
    topv = np.take_along_axis(logits, top2, axis=1)              # (S, 2)
    ex = np.exp(topv - topv.max(axis=1, keepdims=True))
    gsm = ex / ex.sum(axis=1, keepdims=True)
    gates = np.zeros((S, E), dtype=np.float32)
    np.put_along_axis(gates, top2, gsm.astype(np.float32), axis=1)

    sel = np.zeros((S, E), dtype=bool)
    np.put_along_axis(sel, top2, True, axis=1)
    toks = [np.nonzero(sel[:, e])[0] for e in range(E)]
    maxcnt = max(len(t) for t in toks)

    blocks = _plan_blocks(maxcnt)
    C = sum(blocks)
    offs = [sum(blocks[:i]) for i in range(len(blocks))]
    nc = _get_nc(blocks)

    xfT16 = np.ascontiguousarray(xf.T).astype(ml_dtypes.bfloat16)  # [D, S]
    in_maps = []
    for c in range(E):
        tp = np.zeros(C, dtype=np.int64)
        tp[: len(toks[c])] = toks[c]
        im = {
            # w1f[p, f, dt] = w1[c][f, dt*128+p]
            "w1f": np.ascontiguousarray(
                w1[c].astype(ml_dtypes.bfloat16).reshape(F, NTD, 128).transpose(2, 0, 1)),
            # w2f[p, d, ft] = w2[c][d, ft*128+p]
            "w2f": np.ascontiguousarray(
                w2[c].astype(ml_dtypes.bfloat16).reshape(D, NTF, 128).transpose(2, 0, 1)),
            "b1r": np.ascontiguousarray(b1[c].reshape(NTF, 128).T),
            "b2r": np.ascontiguousarray(b2[c].reshape(NTD, 128).T),
        }
        for b in range(len(blocks)):
            g = xfT16[:, tp[offs[b] : offs[b] + blocks[b]]]       # [D, TB]
            im[f"xg{b}"] = np.ascontiguousarray(
                g.reshape(NTD, 128, blocks[b]).transpose(1, 0, 2))
        in_maps.append(im)

    res = run_bass_kernel_spmd(nc, in_maps, core_ids=list(range(8)))
    global LAST_RES
    LAST_RES = res

    out = np.zeros((S, D), dtype=np.float32)
    for c in range(E):
        cnt = len(toks[c])
        yt = np.concatenate(
            [np.asarray(res.results[c][f"yg{b}"]).astype(np.float32)
             .transpose(1, 0, 2).reshape(D, blocks[b])
             for b in range(len(blocks))], axis=1)                # [D, C]
        out[toks[c]] += yt[:, :cnt].T * gates[toks[c], c][:, None]
    return out.reshape(B, T, D)


# revision 14
# speedup vs baseline: 1.3167x; 1.0057x over previous
"""MoE positionwise FFN (top-2 of 8 experts) on 8 TRN2 NeuronCores.

Strategy: expert-parallel, host-routed. The router (logits -> top-2 ->
softmax gates) is exact fp32 on host (as is the final scatter-add
combine, matching the reference semantics). Each core owns one expert:
the host gathers that expert's routed tokens into a compact bf16
input, and the device kernel is a pure dense FFN:

    h = relu(w1 @ x + b1)   (D -> F)
    y = w2 @ h + b2         (F -> D)

over C tokens in blocks (first block largest so block-0 compute
consumes the weight stream slower than DMA supplies it), weight-
stationary bf16 matmuls, fp32 PSUM accumulation.

All device inputs are pre-permuted on host into the exact SBUF layout
so every DMA is a contiguous slice (>=8KB per-partition runs, full HBM
bandwidth): w1 as [128, F, NTD] (f-sliced chunks stream while block-0
computes), w2 as [128, D, NTF] (d-sliced), x per block as [128, NTD,
TB]. Weight chunks alternate across both HWDGE rings (sync + scalar).
The matmul lhsT reads these with a small free-dim stride, which
LDWEIGHTS tolerates (stays hidden behind the matmuls). The PE is
pre-warmed so the HAM clock gate is at 8/8 when real matmuls start.

Self-contained: hardcodes shapes for B=2,T=2048,D=1024,F=4096,E=8,K=2.
"""
import math

import numpy as np
import ml_dtypes

S = 4096
D = 1024
F = 4096
E = 8
NTD = D // 128   # 8 d-tiles
NTF = F // 128   # 32 f-tiles

_cache: dict = {}
LAST_RES = None


def _plan_blocks(maxcnt: int):
    C = max(192, int(math.ceil(maxcnt / 16)) * 16)
    blocks = []
    rem = C
    while rem > 896:
        blocks.append(448)
        rem -= 448
    if rem > 448:
        s1 = int(math.ceil(rem / 2 / 16)) * 16
        blocks.extend([s1, rem - s1])
    elif rem > 0:
        blocks.append(rem)
    return tuple(blocks)


def _build(blocks: tuple):
    import concourse.bacc as bacc
    import concourse.tile as tile
    import concourse.mybir as mybir
    from concourse.tile import add_dep_helper

    f32 = mybir.dt.float32
    bf16 = mybir.dt.bfloat16
    ACT = mybir.ActivationFunctionType

    NB = len(blocks)

    nc = bacc.Bacc("TRN2", target_bir_lowering=False, debug=False, num_devices=8)

    xg_d = [nc.dram_tensor(f"xg{b}", [128, NTD, blocks[b]], bf16,
                           kind="ExternalInput") for b in range(NB)]
    w1_d = nc.dram_tensor("w1f", [128, F, NTD], bf16, kind="ExternalInput")
    w2_d = nc.dram_tensor("w2f", [128, D, NTF], bf16, kind="ExternalInput")
    b1_d = nc.dram_tensor("b1r", [128, NTF], f32, kind="ExternalInput")
    b2_d = nc.dram_tensor("b2r", [128, NTD], f32, kind="ExternalInput")
    yg_d = [nc.dram_tensor(f"yg{b}", [128, NTD, blocks[b]], bf16,
                           kind="ExternalOutput") for b in range(NB)]

    with tile.TileContext(nc) as tc:
        with (
            tc.tile_pool(name="wpool", bufs=1) as wpool,
            tc.tile_pool(name="xr", bufs=1) as xr,
            tc.tile_pool(name="small", bufs=1) as small,
            tc.tile_pool(name="hpool", bufs=1) as hpool,
            tc.tile_pool(name="ypool", bufs=2) as ypool,
            tc.tile_pool(name="psH", bufs=4, space="PSUM") as psH,
            tc.tile_pool(name="psY", bufs=3, space="PSUM") as psY,
            tc.tile_pool(name="psW", bufs=1, space="PSUM") as psW,
        ):
            # ---- PE pre-warm: trip the HAM activity window during the
            # initial DMA so real matmuls start at 2.4 GHz.
            junk = small.tile([128, 256], bf16)
            nc.vector.memset(junk[:], 0.0)
            wps = psW.tile([128, 256], f32)
            for _ in range(28):
                nc.tensor.matmul(wps[:], lhsT=junk[:, 0:128], rhs=junk[:],
                                 start=True, stop=True)

            # ---- DMA streams. Everything big rides the SP (sync) HWDGE
            # ring, depth-2 pipelined: each DMA gets a scheduling edge to
            # the previous one (issue order) and a completion dep on the
            # one before that, so the ring streams back-to-back (receipt
            # latency overlapped) with at most 2 in flight. The ACT
            # (scalar) engine carries ONLY the two dep-free bias DMAs —
            # stream DMAs there would head-of-line-block the activations
            # behind their completion waits, stalling PSUM drain and the
            # PE with it.
            ring = []

            def put(fn):
                d = fn()
                if ring:
                    add_dep_helper(d.ins, ring[-1].ins, sync=False,
                                   reason="ring order")
                if len(ring) >= 3:
                    add_dep_helper(d.ins, ring[-3].ins, sync=True,
                                   reason="ring depth-3")
                ring.append(d)
                return d

            b1_sb = small.tile([128, NTF], f32)
            nc.scalar.dma_start(out=b1_sb[:], in_=b1_d[:, :])
            b2_sb = small.tile([128, NTD], f32)
            nc.scalar.dma_start(out=b2_sb[:], in_=b2_d[:, :])

            xg = [xr.tile([128, NTD, blocks[b]], bf16, tag=f"xg{b}",
                          name=f"xg{b}")
                  for b in range(NB)]
            put(lambda: nc.sync.dma_start(out=xg[0][:], in_=xg_d[0][:, :, :]))

            w1_sb = wpool.tile([128, F, NTD], bf16)   # w1_sb[p,f,dt] = w1[f, dt*128+p]
            w2_sb = wpool.tile([128, D, NTF], bf16)   # w2_sb[p,d,ft] = w2[d, ft*128+p]
            # fine-grained first chunks so block-0 ft=0 can start early
            w1cuts = [0, 128, 256, 512, 1024, 1536, 2048, 2560, 3072, 3584, F]
            for lo, hi in zip(w1cuts, w1cuts[1:]):
                put(lambda lo=lo, hi=hi: nc.sync.dma_start(
                    out=w1_sb[:, lo:hi, :], in_=w1_d[:, lo:hi, :]))
            DC = D // 8
            for i in range(8):
                put(lambda i=i: nc.sync.dma_start(
                    out=w2_sb[:, i * DC : (i + 1) * DC, :],
                    in_=w2_d[:, i * DC : (i + 1) * DC, :]))
            for b in range(1, NB):
                put(lambda b=b: nc.sync.dma_start(
                    out=xg[b][:], in_=xg_d[b][:, :, :]))

            # ---- FFN over token blocks ---------------------------------
            for blk, TB in enumerate(blocks):
                xg_blk = xg[blk]
                h_sb = hpool.tile([128, NTF, max(blocks)], bf16, tag="h")
                for ft in range(NTF):
                    hp = psH.tile([128, TB], f32, tag="hps")
                    for dt in range(NTD):
                        nc.tensor.matmul(
                            hp[:],
                            lhsT=w1_sb[:, ft * 128 : (ft + 1) * 128, dt],
                            rhs=xg_blk[:, dt, :],
                            start=(dt == 0),
                            stop=(dt == NTD - 1),
                        )
                    nc.scalar.activation(out=h_sb[:, ft, 0:TB], in_=hp[:], func=ACT.Relu,
                                         bias=b1_sb[:, ft : ft + 1], scale=1.0)
                y_blk = ypool.tile([128, NTD, TB], bf16, tag="y")
                for dt in range(NTD):
                    yp = psY.tile([128, TB], f32, tag="yps")
                    for ft in range(NTF):
                        nc.tensor.matmul(
                            yp[:],
                            lhsT=w2_sb[:, dt * 128 : (dt + 1) * 128, ft],
                            rhs=h_sb[:, ft, 0:TB],
                            start=(ft == 0),
                            stop=(ft == NTF - 1),
                        )
                    nc.vector.tensor_scalar_add(y_blk[:, dt, :], yp[:], b2_sb[:, dt : dt + 1])
                    if blk == len(blocks) - 1:
                        put(lambda blk=blk, dt=dt, y_blk=y_blk: nc.sync.dma_start(
                            out=yg_d[blk][:, dt, :], in_=y_blk[:, dt, :]))
                if blk < len(blocks) - 1:
                    put(lambda blk=blk, y_blk=y_blk: nc.sync.dma_start(
                        out=yg_d[blk][:, :, :], in_=y_blk[:]))

    nc.compile()
    return nc


def _get_nc(blocks: tuple):
    if blocks not in _cache:
        _cache[blocks] = _build(blocks)
    return _cache[blocks]


def kernel(x, gate_w, w1, b1, w2, b2, k):
    from concourse.bass_utils import run_bass_kernel_spmd

    assert int(k) == 2
    x = np.asarray(x, dtype=np.float32)
    gate_w = np.asarray(gate_w, dtype=np.float32)
    w1 = np.asarray(w1, dtype=np.float32)
    b1 = np.asarray(b1, dtype=np.float32)
    w2 = np.asarray(w2, dtype=np.float32)
    b2 = np.asarray(b2, dtype=np.float32)
    B, T, _ = x.shape
    xf = x.reshape(S, D)

    # Router (exact fp32, matching the reference's top-2 renormalized
    # softmax; gates applied host-side during the merge).
    logits = xf @ gate_w.T
    top2 = np.argpartition(-logits, 2, axis=1)[:, :2]
    topv = np.take_along_axis(logits, top2, axis=1)              # (S, 2)
    ex = np.exp(topv - topv.max(axis=1, keepdims=True))
    gsm = ex / ex.sum(axis=1, keepdims=True)
    gates = np.zeros((S, E), dtype=np.float32)
    np.put_along_axis(gates, top2, gsm.astype(np.float32), axis=1)

    sel = np.zeros((S, E), dtype=bool)
    np.put_along_axis(sel, top2, True, axis=1)
    toks = [np.nonzero(sel[:, e])[0] for e in range(E)]
    maxcnt = max(len(t) for t in toks)

    blocks = _plan_blocks(maxcnt)
    C = sum(blocks)
    offs = [sum(blocks[:i]) for i in range(len(blocks))]
    nc = _get_nc(blocks)

    xfT16 = np.ascontiguousarray(xf.T).astype(ml_dtypes.bfloat16)  # [D, S]
    in_maps = []
    for c in range(E):
        tp = np.zeros(C, dtype=np.int64)
        tp[: len(toks[c])] = toks[c]
        im = {
            # w1f[p, f, dt] = w1[c][f, dt*128+p]
            "w1f": np.ascontiguousarray(
                w1[c].astype(ml_dtypes.bfloat16).reshape(F, NTD, 128).transpose(2, 0, 1)),
            # w2f[p, d, ft] = w2[c][d, ft*128+p]
            "w2f": np.ascontiguousarray(
                w2[c].astype(ml_dtypes.bfloat16).reshape(D, NTF, 128).transpose(2, 0, 1)),
            "b1r": np.ascontiguousarray(b1[c].reshape(NTF, 128).T),
            "b2r": np.ascontiguousarray(b2[c].reshape(NTD, 128).T),
        }
        for b in range(len(blocks)):
            g = xfT16[:, tp[offs[b] : offs[b] + blocks[b]]]       # [D, TB]
            im[f"xg{b}"] = np.ascontiguousarray(
                g.reshape(NTD, 128, blocks[b]).transpose(1, 0, 2))
        in_maps.append(im)

    res = run_bass_kernel_spmd(nc, in_maps, core_ids=list(range(8)))
    global LAST_RES
    LAST_RES = res

    out = np.zeros((S, D), dtype=np.float32)
    for c in range(E):
        cnt = len(toks[c])
        yt = np.concatenate(
            [np.asarray(res.results[c][f"yg{b}"]).astype(np.float32)
             .transpose(1, 0, 2).reshape(D, blocks[b])
             for b in range(len(blocks))], axis=1)                # [D, C]
        out[toks[c]] += yt[:, :cnt].T * gates[toks[c], c][:, None]
    return out.reshape(B, T, D)
